# revision 1
# baseline (speedup 1.0000x reference)
"""MoE (top-2 of 8 experts, GELU MLP) on 8 Trainium2 NeuronCores.

Strategy (expert-parallel, per sharding hint):
  Launch 1 (gate, data-parallel): each core takes a 1024-token shard and
    computes per-token combine weights comb[t, e] (softmax over the top-2
    expert logits, scattered to the selected experts) fully on device.
  Host glue: build per-expert token index lists from comb (pure
    gather/scatter data movement), gather x columns per expert.
  Launch 2 (experts): core e runs its expert's GELU MLP over the tokens
    routed to it (padded to a common capacity C), in bf16 with fp32 PSUM
    accumulation, scales by the combine weight, returns y^T per expert.
  Host glue: scatter-add the (disjoint-per-expert) rows into the output.
"""

import sys

import numpy as np

try:
    import concourse.bass as bass  # noqa: F401
except ImportError:  # container default location of the concourse repo
    sys.path.insert(0, "/opt/trn_rl_repo")

import concourse.bass as bass
import concourse.tile as tile
from concourse import bacc, mybir
from concourse.bass_utils import run_bass_kernel_spmd

F32 = mybir.dt.float32
BF16 = mybir.dt.bfloat16
AF = mybir.ActivationFunctionType
ALU = mybir.AluOpType

E = 8          # experts
D = 1024       # d_model
F = 2048       # d_ff
T = 8192       # tokens (4*2048)
NCORES = 8
TSHARD = T // NCORES
P = 128


def _new_nc():
    return bacc.Bacc("TRN2", target_bir_lowering=False, debug=False,
                     num_devices=NCORES)


def build_gate_nc(repeat: int = 1):
    """Per core: xt [D, TSHARD] f32 (x^T token shard), wg [D, E], bgb [P, E]
    (gate bias broadcast across partitions) -> comb [TSHARD, E] f32."""
    nc = _new_nc()
    xt = nc.dram_tensor("xt", [D, TSHARD], F32, kind="ExternalInput").ap()
    wg = nc.dram_tensor("wg", [D, E], F32, kind="ExternalInput").ap()
    bgb = nc.dram_tensor("bgb", [P, E], F32, kind="ExternalInput").ap()
    comb = nc.dram_tensor("comb", [TSHARD, E], F32, kind="ExternalOutput").ap()

    KD = D // P          # 8 contraction tiles
    NT = TSHARD // P     # 8 token tile groups

    def bc(ap):  # [P, NT] -> [P, NT, E] broadcast (step-0 inner axis)
        return ap.rearrange("p (g a) -> p g a", a=1).broadcast_to((P, NT, E))

    with tile.TileContext(nc) as tc:
        with (
            tc.tile_pool(name="res", bufs=1) as res,
            tc.tile_pool(name="io", bufs=3) as io,
            tc.tile_pool(name="tmp", bufs=2) as tmp,
            tc.tile_pool(name="psum", bufs=2, space="PSUM") as psum,
        ):
            for _ in range(repeat):
                wgsb = res.tile([P, KD, E], F32, tag="wgsb")
                nc.sync.dma_start(wgsb[:], wg.rearrange("(ko ki) e -> ki ko e", ki=P))
                bgsb = res.tile([P, E], F32, tag="bgsb")
                nc.sync.dma_start(bgsb[:], bgb[:])

                xt3 = xt.rearrange("(ko ki) n -> ki ko n", ki=P)
                ps = psum.tile([P, NT, E], F32, tag="ps")
                xsbs = []
                for k in range(KD):
                    xsb = io.tile([P, TSHARD], F32, tag=f"xsb_{k}")
                    nc.sync.dma_start(xsb[:], xt3[:, k, :])
                    xsbs.append(xsb)
                for t in range(NT):
                    for k in range(KD):
                        nc.tensor.matmul(ps[:, t, :],
                                         xsbs[k][:, t * P:(t + 1) * P],
                                         wgsb[:, k, :],
                                         start=(k == 0), stop=(k == KD - 1))

                # batched top-2 + softmax epilogue over all NT groups at once
                L = tmp.tile([P, NT, E], F32, tag="L")
                nc.vector.tensor_tensor(
                    L[:], ps[:],
                    bgsb.rearrange("p (a e) -> p a e", a=1).broadcast_to(
                        (P, NT, E)),
                    op=ALU.add)
                mx1 = tmp.tile([P, NT], F32, tag="mx1")
                nc.vector.reduce_max(mx1[:], L[:], axis=mybir.AxisListType.X)
                m1 = tmp.tile([P, NT, E], F32, tag="m1")
                nc.vector.tensor_tensor(m1[:], L[:], bc(mx1), op=ALU.is_equal)
                l2 = tmp.tile([P, NT, E], F32, tag="l2")
                nc.vector.scalar_tensor_tensor(
                    l2[:], m1[:], -1e30, L[:], op0=ALU.mult, op1=ALU.add)
                mx2 = tmp.tile([P, NT], F32, tag="mx2")
                nc.vector.reduce_max(mx2[:], l2[:], axis=mybir.AxisListType.X)
                m2 = tmp.tile([P, NT, E], F32, tag="m2")
                nc.vector.tensor_tensor(m2[:], l2[:], bc(mx2), op=ALU.is_equal)
                # w1 = 1/(1+exp(mx2-mx1)), w2 = 1-w1  (softmax over top-2)
                dl = tmp.tile([P, NT], F32, tag="dl")
                nc.vector.tensor_sub(dl[:], mx2[:], mx1[:])
                ex = tmp.tile([P, NT], F32, tag="ex")
                nc.scalar.activation(ex[:], dl[:], AF.Exp)
                s = tmp.tile([P, NT], F32, tag="s")
                nc.vector.tensor_scalar_add(s[:], ex[:], 1.0)
                w1 = tmp.tile([P, NT], F32, tag="w1")
                nc.vector.reciprocal(w1[:], s[:])
                w2 = tmp.tile([P, NT], F32, tag="w2")
                nc.vector.tensor_mul(w2[:], ex[:], w1[:])

                c1 = tmp.tile([P, NT, E], F32, tag="c1")
                nc.vector.tensor_tensor(c1[:], m1[:], bc(w1), op=ALU.mult)
                c2 = tmp.tile([P, NT, E], F32, tag="c2")
                nc.vector.tensor_tensor(c2[:], m2[:], bc(w2), op=ALU.mult)
                cmb = tmp.tile([P, NT, E], F32, tag="cmb")
                nc.vector.tensor_add(cmb[:], c1[:], c2[:])
                nc.sync.dma_start(
                    comb.rearrange("(g p) e -> p g e", p=P), cmb[:])
    nc.compile()
    return nc


def build_expert_nc(C: int, repeat: int = 1, ntile: int = 512,
                    parts: str = "full"):
    """Per core: one expert's GELU MLP over C (padded) routed tokens.

    xgt [D, C] f32 gathered x^T; wb [P, C] f32 combine weight broadcast
    across partitions; w1 [D, F]; b1c [P, F//P]; w2 [F, D]; b2c [P, D//P]
    -> yt [D, C] f32 where yt[:, j] = wb[j] * (gelu(x_j @ W1 + b1) @ W2 + b2).
    """
    assert C % P == 0
    nc = _new_nc()
    xgt = nc.dram_tensor("xgt", [D, C], F32, kind="ExternalInput").ap()
    wb = nc.dram_tensor("wb", [P, C], F32, kind="ExternalInput").ap()
    w1 = nc.dram_tensor("w1", [D, F], F32, kind="ExternalInput").ap()
    b1c = nc.dram_tensor("b1c", [P, F // P], F32, kind="ExternalInput").ap()
    w2 = nc.dram_tensor("w2", [F, D], F32, kind="ExternalInput").ap()
    b2c = nc.dram_tensor("b2c", [P, D // P], F32, kind="ExternalInput").ap()
    yt = nc.dram_tensor("yt", [D, C], F32, kind="ExternalOutput").ap()

    KD = D // P    # 8  k-tiles for x @ W1
    KF = F // P    # 16 k-tiles for h @ W2
    MF = F // P    # 16 dff output tiles
    MD = D // P    # 8  dmodel output tiles
    NTILE = ntile
    ntok = [(n0, min(NTILE, C - n0)) for n0 in range(0, C, NTILE)]
    scale = NTILE // 512  # keep SBUF/PSUM footprint constant across ntile

    with tile.TileContext(nc) as tc:
        with (
            tc.tile_pool(name="res", bufs=1) as res,
            tc.tile_pool(name="stg", bufs=2) as stg,
            tc.tile_pool(name="hbuf", bufs=max(1, 2 // scale)) as hbuf,
            tc.tile_pool(name="obuf", bufs=max(2, 4 // scale)) as obuf,
            tc.tile_pool(name="psum", bufs=8 // scale, space="PSUM") as psum,
        ):
            do_io = parts in ("full", "io")
            do_mm = parts in ("full", "compute")
            for _ in range(repeat):
                b1sb = res.tile([P, F // P], F32, tag="b1sb")
                nc.sync.dma_start(b1sb[:], b1c[:])
                b2sb = res.tile([P, D // P], F32, tag="b2sb")
                nc.sync.dma_start(b2sb[:], b2c[:])

                # resident bf16 weights and activations (cast on load).
                # Order by first use: (x[k], W1[k]) pairs feed phase 1
                # immediately; W2 and wb are only needed once phase 2 starts.
                w1sb, w2sb, xsb = [], [], []
                for k in range(KD):
                    xk = res.tile([P, C], BF16, tag=f"x_{k}")
                    w1k = res.tile([P, F], BF16, tag=f"w1_{k}")
                    if do_io:
                        xstg = stg.tile([P, C], F32, tag="stg_x")
                        nc.sync.dma_start(xstg[:], xgt[k * P:(k + 1) * P, :])
                        nc.gpsimd.tensor_copy(xk[:], xstg[:])
                        w1stg = stg.tile([P, F], F32, tag="stg_a")
                        nc.sync.dma_start(w1stg[:], w1[k * P:(k + 1) * P, :])
                        nc.vector.tensor_copy(w1k[:], w1stg[:])
                    xsb.append(xk)
                    w1sb.append(w1k)
                for k in range(KF):
                    w2k = res.tile([P, D], BF16, tag=f"w2_{k}")
                    if do_io:
                        w2stg = stg.tile([P, D], F32, tag="stg_b")
                        nc.sync.dma_start(w2stg[:], w2[k * P:(k + 1) * P, :])
                        nc.scalar.activation(w2k[:], w2stg[:], AF.Copy)
                    w2sb.append(w2k)
                wbsb = res.tile([P, C], F32, tag="wbsb")
                nc.sync.dma_start(wbsb[:], wb[:])

                for n0, nn in (ntok if do_mm else []):
                    hs = []
                    for mf in range(MF):
                        ps = psum.tile([P, NTILE], F32, tag="ps")
                        for k in range(KD):
                            nc.tensor.matmul(
                                ps[:, :nn],
                                w1sb[k][:, mf * P:(mf + 1) * P],
                                xsb[k][:, n0:n0 + nn],
                                start=(k == 0), stop=(k == KD - 1))
                        h = hbuf.tile([P, NTILE], BF16, tag=f"h_{mf}")
                        nc.scalar.activation(h[:, :nn], ps[:, :nn],
                                             AF.Gelu_apprx_tanh,
                                             bias=b1sb[:, mf:mf + 1])
                        hs.append(h)
                    for md in range(MD):
                        ps2 = psum.tile([P, NTILE], F32, tag="ps")
                        for k in range(KF):
                            nc.tensor.matmul(
                                ps2[:, :nn],
                                w2sb[k][:, md * P:(md + 1) * P],
                                hs[k][:, :nn],
                                start=(k == 0), stop=(k == KF - 1))
                        # yw = (y + b2) * w  in one DVE op
                        yw = obuf.tile([P, NTILE], F32, tag="yw")
                        nc.vector.scalar_tensor_tensor(
                            yw[:, :nn], ps2[:, :nn], b2sb[:, md:md + 1],
                            wbsb[:, n0:n0 + nn], op0=ALU.add, op1=ALU.mult)
                        nc.sync.dma_start(yt[md * P:(md + 1) * P, n0:n0 + nn],
                                          yw[:, :nn])
    nc.compile()
    return nc


def _run(nc, in_maps):
    res = run_bass_kernel_spmd(nc, in_maps, core_ids=list(range(NCORES)))
    return res.results


def gate_in_maps(xT, Wg, bg):
    bgb = np.ascontiguousarray(np.broadcast_to(bg, (P, E)), dtype=np.float32)
    wg = np.ascontiguousarray(Wg, dtype=np.float32)
    return [
        {"xt": np.ascontiguousarray(xT[:, c * TSHARD:(c + 1) * TSHARD]),
         "wg": wg, "bgb": bgb}
        for c in range(NCORES)
    ]


def routing_from_comb(comb):
    idxs = [np.nonzero(comb[:, e])[0] for e in range(E)]
    maxn = max(len(i) for i in idxs)
    C = max(((maxn + P - 1) // P) * P, P)
    return idxs, C


def expert_in_maps(xT, comb, idxs, C, W1, b1, W2, b2):
    in_maps = []
    for e in range(E):
        idx = idxs[e]
        n = len(idx)
        xgt = np.zeros((D, C), np.float32)
        xgt[:, :n] = xT[:, idx]
        wbe = np.zeros((P, C), np.float32)
        wbe[:, :n] = comb[idx, e][None, :]
        in_maps.append({
            "xgt": xgt,
            "wb": wbe,
            "w1": np.ascontiguousarray(W1[e], dtype=np.float32),
            "b1c": np.ascontiguousarray(
                b1[e].reshape(F // P, P).T, dtype=np.float32),
            "w2": np.ascontiguousarray(W2[e], dtype=np.float32),
            "b2c": np.ascontiguousarray(
                b2[e].reshape(D // P, P).T, dtype=np.float32),
        })
    return in_maps


def combine_outputs(outs, idxs, x_shape):
    out = np.zeros((T, D), np.float32)
    for e in range(E):
        idx = idxs[e]
        out[idx] += outs[e]["yt"][:, :len(idx)].T
    return out.reshape(x_shape)


def kernel(x, Wg, bg, W1, b1, W2, b2):
    x = np.asarray(x, dtype=np.float32)
    Wg = np.asarray(Wg, dtype=np.float32)
    bg = np.asarray(bg, dtype=np.float32)
    W1 = np.asarray(W1, dtype=np.float32)
    b1 = np.asarray(b1, dtype=np.float32)
    W2 = np.asarray(W2, dtype=np.float32)
    b2 = np.asarray(b2, dtype=np.float32)

    xf = x.reshape(T, D)
    xT = np.ascontiguousarray(xf.T)

    nc_g = build_gate_nc()
    combs = _run(nc_g, gate_in_maps(xT, Wg, bg))
    comb = np.concatenate([r["comb"] for r in combs], axis=0)

    idxs, C = routing_from_comb(comb)
    nc_e = build_expert_nc(C)
    outs = _run(nc_e, expert_in_maps(xT, comb, idxs, C, W1, b1, W2, b2))
    return combine_outputs(outs, idxs, x.shape)

